# revision 35
# baseline (speedup 1.0000x reference)
"""Trainium2 Bass kernel for nn_OFDM_channel.

Math: the reference's ifft -> CP -> L-tap causal conv -> CP-strip -> fft
chain is exactly a per-symbol circular convolution (conv tail L-1=7 fits
inside the CP K=16), so in the frequency domain the whole model collapses
to:

    out[n,p,j,f] = H[n,p,f] * X[n,p,j,f] + c_n * FFT64(noise[n,p,j,16:80])
    H[n,p,:]     = FFT64(sqrt(prof/2) * cof_unit[n,p], zero-padded)
    X[.,.,0,:]   = alpha[n,p] * pilot_raw (complex), X[.,.,1+s,:] = x
    alpha[n,p]   = sqrt(PWR/2) / sqrt(mean(pilot_raw^2))
    noise_pwr    = PWR / (M * 10^(SNR/10)),  c_n = sqrt(noise_pwr / 2)

(verified to rel err ~1.5e-7 against the fp32 jax reference).

Sharding: pure data-parallel over the leading batch dim N (1024) across
8 NeuronCores, 128 n-values per core.  All layouts keep (n,p) rows on
SBUF partitions and interleaved (freq, re/im) on the free axis, so every
HBM transfer is >= 512B-contiguous per row.

Per core (512 (n,p) rows = 4 partition-tiles of 128):
  - H, Hre-dup, HimS-dup via one K=16 matmul each against constant
    interleaved-DFT matrices (taps transposed on TensorE).
  - noise FFT: TensorE transpose per symbol -> PSUM -> SBUF -> K=128
    matmul against the constant noise-DFT matrix (c_n folded in).
  - H*X via two VectorE tensor_tensor passes with stride-0 broadcast
    APs (H broadcast over the 13 symbols), a lane-swap add, and a final
    add of the PSUM noise-FFT results.
"""

import numpy as np
from contextlib import ExitStack

import concourse.bass as bass
import concourse.bacc as bacc
import concourse.tile as tile
import concourse.mybir as mybir
from concourse.bass_utils import run_bass_kernel_spmd

# problem constants (hardcoded per contract)
N, P, S, M, K, L = 1024, 4, 12, 64, 16, 8
SNR, PWR = 20.0, 1.0
NOISE_PWR = PWR / (M * 10.0 ** (0.1 * SNR))  # 1/6400
C_N = float(np.sqrt(NOISE_PWR / 2.0))

NCORES = 8
NSH = N // NCORES            # 128 n per core
R = NSH * P                  # 512 (n,p) rows per core
RT = R // 128                # 4 row-tiles of 128 rows
NSYM = S + 1                 # 13 symbols (pilot + S payload)
W = 2 * M                    # 128 floats per symbol (interleaved re/im)
XW = NSYM * W                # 1664 floats per row across all symbols
MK2 = 2 * (M + K)            # 160 floats per noise symbol row

F32 = mybir.dt.float32
AF = mybir.ActivationFunctionType
ALU = mybir.AluOpType


def _constants():
    """Interleaved-complex DFT matrices.

    Layout convention: a complex vector z is stored as interleaved floats
    [re0, im0, re1, im1, ...].  A matmul  out = zT @ W2  with
    W2[(2m, 2m+1), (2f, 2f+1)] blocks implementing complex multiply by
    e^{-i phi} computes the DFT in interleaved form.
    """
    f = np.arange(M)
    # H = FFT64 of sqrt(prof/2)-scaled taps (L=8 taps, zero padded)
    phT = 2.0 * np.pi * np.outer(np.arange(L), f) / M     # (L, M)
    prof = np.exp(-np.arange(L, dtype=np.float64) / (L // 2))
    prof = prof / prof.sum()
    s = np.sqrt(prof / 2.0)[:, None]
    cT, sT = np.cos(phT), np.sin(phT)

    w2h = np.zeros((2 * L, W), np.float64)
    w2h[0::2, 0::2] = s * cT      # re <- u
    w2h[1::2, 0::2] = s * sT      # re <- v
    w2h[0::2, 1::2] = -s * sT     # im <- u
    w2h[1::2, 1::2] = s * cT      # im <- v

    # Hre duplicated into both lanes: [Hre, Hre] per f
    w2h_re = np.zeros((2 * L, W), np.float64)
    w2h_re[0::2, 0::2] = s * cT
    w2h_re[1::2, 0::2] = s * sT
    w2h_re[0::2, 1::2] = s * cT
    w2h_re[1::2, 1::2] = s * sT

    # HimS: even lane = +Him, odd lane = -Him  (Him = -u*sin + v*cos)
    w2h_ims = np.zeros((2 * L, W), np.float64)
    w2h_ims[0::2, 0::2] = -s * sT
    w2h_ims[1::2, 0::2] = s * cT
    w2h_ims[0::2, 1::2] = s * sT
    w2h_ims[1::2, 1::2] = -s * cT

    # noise FFT (64-point) with c_n folded in
    phM = 2.0 * np.pi * np.outer(np.arange(M), f) / M     # (M, M)
    cM, sM = np.cos(phM), np.sin(phM)
    w2n = np.zeros((W, W), np.float64)
    w2n[0::2, 0::2] = C_N * cM
    w2n[1::2, 0::2] = C_N * sM
    w2n[0::2, 1::2] = -C_N * sM
    w2n[1::2, 1::2] = C_N * cM

    ident = np.eye(128, dtype=np.float32)
    w2h3 = np.hstack([w2h, w2h_re, w2h_ims])   # (16, 384)
    return {
        "w2h3_d": w2h3.astype(np.float32),
        "w2n_d": w2n.astype(np.float32),
        "ident_d": ident,
    }


def _bcast13(ap):
    """[128, W] AP -> broadcast view [128, 13, W] (stride-0 symbol axis)."""
    return bass.AP(ap.tensor, ap.offset, [ap.ap[0], [0, NSYM], ap.ap[1]])


def build_nc():
    """Trace + compile the per-core kernel. Same program for all 8 cores."""
    nc = bacc.Bacc("TRN2", target_bir_lowering=False, debug=False)

    x_d = nc.dram_tensor("x_d", [R, S * W], F32, kind="ExternalInput").ap()
    pilot_d = nc.dram_tensor("pilot_d", [R, W], F32, kind="ExternalInput").ap()
    cofT_d = nc.dram_tensor("cofT_d", [2 * L, R], F32, kind="ExternalInput").ap()
    noise_d = nc.dram_tensor("noise_d", [R, NSYM, MK2], F32, kind="ExternalInput").ap()
    w2h3_d = nc.dram_tensor("w2h3_d", [2 * L, 3 * W], F32, kind="ExternalInput").ap()
    w2n_d = nc.dram_tensor("w2n_d", [W, W], F32, kind="ExternalInput").ap()
    ident_d = nc.dram_tensor("ident_d", [128, 128], F32, kind="ExternalInput").ap()

    sig_o = nc.dram_tensor("sig_o", [R, S * W], F32, kind="ExternalOutput").ap()
    pil_o = nc.dram_tensor("pil_o", [R, W], F32, kind="ExternalOutput").ap()
    h_o = nc.dram_tensor("h_o", [R, W], F32, kind="ExternalOutput").ap()

    with tile.TileContext(nc) as tc, ExitStack() as ctx:
        const = ctx.enter_context(tc.tile_pool(name="const", bufs=1))
        xpool = ctx.enter_context(tc.tile_pool(name="xpool", bufs=3))
        npool = ctx.enter_context(tc.tile_pool(name="npool", bufs=3))
        mpool = ctx.enter_context(tc.tile_pool(name="mpool", bufs=3))
        qpool = ctx.enter_context(tc.tile_pool(name="qpool", bufs=3))
        spool = ctx.enter_context(tc.tile_pool(name="spool", bufs=3))
        ps_big = ctx.enter_context(tc.tile_pool(name="ps_big", bufs=2, space="PSUM"))
        ps_xt = ctx.enter_context(tc.tile_pool(name="ps_xt", bufs=2, space="PSUM"))

        w2h3_t = const.tile([2 * L, 3 * W], F32)
        nc.sync.dma_start(w2h3_t[:], w2h3_d)
        w2n_t = const.tile([W, W], F32)
        nc.sync.dma_start(w2n_t[:], w2n_d)
        ident_t = const.tile([128, 128], F32)
        nc.sync.dma_start(ident_t[:], ident_d)

        # ---- channel taps: host-transposed [16, 512], one small DMA ---
        cofT_all = const.tile([2 * L, R], F32)
        nc.sync.dma_start(cofT_all[:], cofT_d)

        for rt in range(RT):
            r0 = rt * 128
            rsl = slice(r0, r0 + 128)

            # ---- loads -------------------------------------------------
            xfull = xpool.tile([128, XW], F32, tag="xfull")
            nc.sync.dma_start(xfull[:, 0:W], pilot_d[rsl, :])
            nc.sync.dma_start(xfull[:, W:XW], x_d[rsl, :])
            noise_t = npool.tile([128, XW], F32, tag="noise")
            nc.sync.dma_start(noise_t[:, 0:896], noise_d[rsl, 0:7, 2 * K:MK2])
            nc.sync.dma_start(noise_t[:, 896:XW], noise_d[rsl, 7:NSYM, 2 * K:MK2])
            # ---- pilot normalization (alpha) ---------------------------
            sq = spool.tile([128, W], F32, tag="sq")
            ss = spool.tile([128, 1], F32, tag="ss")
            nc.scalar.activation(sq[:], xfull[:, 0:W], AF.Square, accum_out=ss[:])
            st = spool.tile([128, 1], F32, tag="st")
            # alpha = 1/sqrt(sumsq/64)  (= sqrt(PWR/2)/sqrt(mean over 128))
            nc.scalar.activation(st[:], ss[:], AF.Sqrt, scale=1.0 / 64.0)
            al = spool.tile([128, 1], F32, tag="al")
            nc.vector.reciprocal(al[:], st[:])
            nc.vector.tensor_scalar_mul(xfull[:, 0:W], xfull[:, 0:W], al[:])

            # ---- H chain (8-tap FFT via K=16 matmul) -------------------
            h3_ps = ps_xt.tile([128, 3 * W], F32, tag="xt_ps")
            nc.tensor.matmul(h3_ps[:], cofT_all[:, rsl], w2h3_t[:],
                             start=True, stop=True)
            h3_sb = spool.tile([128, 3 * W], F32, tag="h3_sb")
            nc.scalar.copy(h3_sb[:], h3_ps[:])
            nc.sync.dma_start(h_o[rsl, :], h3_sb[:, 0:W])
            hre_sb = h3_sb[:, W:2 * W]
            hims_sb = h3_sb[:, 2 * W:3 * W]

            # ---- noise FFT: transposes (2 psum halves) + matmuls -------
            xt_sb = spool.tile([128, XW], F32, tag="xt_sb")
            for j0, j1 in ((0, 7), (7, NSYM)):
                xt_ps = ps_xt.tile([128, 896], F32, tag="xt_ps")
                for j in range(j0, j1):
                    nc.tensor.transpose(
                        xt_ps[:, (j - j0) * 128:(j - j0 + 1) * 128],
                        noise_t[:, j * W:(j + 1) * W],
                        ident_t[:],
                    )
                nc.scalar.copy(xt_sb[:, j0 * 128:j1 * 128],
                               xt_ps[:, :(j1 - j0) * 128])
            prod_a = ps_big.tile([128, 896], F32, tag="prod")
            prod_b = ps_big.tile([128, 768], F32, tag="prod")
            for j in range(NSYM):
                pp, c0 = (prod_a, 0) if j < 7 else (prod_b, 896)
                nc.tensor.matmul(
                    pp[:, j * W - c0:(j + 1) * W - c0],
                    xt_sb[:, j * 128:(j + 1) * 128],
                    w2n_t[:],
                    start=True, stop=True,
                )

            # ---- H * X (interleaved complex mult, broadcast H over j) --
            m1 = mpool.tile([128, XW], F32, tag="m1")
            q = qpool.tile([128, XW], F32, tag="q")
            xv = xfull[:].rearrange("p (j w) -> p j w", w=W)
            m1v3 = m1[:].rearrange("p (j w) -> p j w", w=W)
            qv3 = q[:].rearrange("p (j w) -> p j w", w=W)
            nc.vector.tensor_tensor(m1v3, xv, _bcast13(hre_sb), ALU.mult)
            nc.vector.tensor_tensor(qv3, xv, _bcast13(hims_sb), ALU.mult)
            # m1 += swap(q):  even lane += q_odd, odd lane += q_even
            m1p = m1[:].rearrange("p (a b) -> p a b", b=2)
            qp = q[:].rearrange("p (a b) -> p a b", b=2)
            nc.vector.tensor_tensor(m1p[:, :, 0], m1p[:, :, 0], qp[:, :, 1], ALU.add)
            nc.vector.tensor_tensor(m1p[:, :, 1], m1p[:, :, 1], qp[:, :, 0], ALU.add)
            # out = m1 + noise_fft, per psum half (reuse q as out buffer)
            nc.vector.tensor_tensor(q[:, 0:896], m1[:, 0:896], prod_a[:], ALU.add)
            nc.sync.dma_start(pil_o[rsl, :], q[:, 0:W])
            nc.sync.dma_start(sig_o[rsl, 0:768], q[:, W:896])
            nc.vector.tensor_tensor(q[:, 896:XW], m1[:, 896:XW], prod_b[:], ALU.add)
            nc.sync.dma_start(sig_o[rsl, 768:S * W], q[:, 896:XW])

    nc.compile()
    return nc


_CACHE = {}


def _get_nc():
    if "nc" not in _CACHE:
        _CACHE["nc"] = build_nc()
        _CACHE["consts"] = _constants()
    return _CACHE["nc"], _CACHE["consts"]


def make_in_maps(x, pilot_raw, cof_unit, noise_unit):
    consts = _get_nc()[1]
    in_maps = []
    for c in range(NCORES):
        sl = slice(c * NSH, (c + 1) * NSH)
        in_maps.append({
            "x_d": np.ascontiguousarray(x[sl]).reshape(R, S * W),
            "pilot_d": np.ascontiguousarray(pilot_raw[sl]).reshape(R, W),
            "cofT_d": np.ascontiguousarray(
                cof_unit[sl].reshape(R, 2 * L).T),
            "noise_d": np.ascontiguousarray(noise_unit[sl]).reshape(R, NSYM, MK2),
            **consts,
        })
    return in_maps


def assemble(results):
    pil = np.concatenate([r["pil_o"] for r in results], axis=0)
    sig = np.concatenate([r["sig_o"] for r in results], axis=0)
    h = np.concatenate([r["h_o"] for r in results], axis=0)
    info_pilot = pil.reshape(N, P, 1, M, 2)
    info_sig = sig.reshape(N, P, S, M, 2)
    h_true = h.reshape(N, P, M, 2)
    return info_pilot, info_sig, h_true, np.float32(NOISE_PWR)


def kernel(x, pilot_raw, cof_unit, noise_unit, _trace=False, _trace_kwargs=None):
    x = np.asarray(x, dtype=np.float32)
    pilot_raw = np.asarray(pilot_raw, dtype=np.float32)
    cof_unit = np.asarray(cof_unit, dtype=np.float32)
    noise_unit = np.asarray(noise_unit, dtype=np.float32)

    nc, _ = _get_nc()
    in_maps = make_in_maps(x, pilot_raw, cof_unit, noise_unit)
    res = run_bass_kernel_spmd(
        nc, in_maps, core_ids=list(range(NCORES)),
        trace=_trace, **(_trace_kwargs or {}),
    )
    out = assemble(res.results)
    if _trace:
        return out, res
    return out


# revision 36
# speedup vs baseline: 1.0249x; 1.0249x over previous
"""Trainium2 Bass kernel for nn_OFDM_channel.

Math: the reference's ifft -> CP -> L-tap causal conv -> CP-strip -> fft
chain is exactly a per-symbol circular convolution (conv tail L-1=7 fits
inside the CP K=16), so in the frequency domain the whole model collapses
to:

    out[n,p,j,f] = H[n,p,f] * X[n,p,j,f] + c_n * FFT64(noise[n,p,j,16:80])
    H[n,p,:]     = FFT64(sqrt(prof/2) * cof_unit[n,p], zero-padded)
    X[.,.,0,:]   = alpha[n,p] * pilot_raw (complex), X[.,.,1+s,:] = x
    alpha[n,p]   = sqrt(PWR/2) / sqrt(mean(pilot_raw^2))
    noise_pwr    = PWR / (M * 10^(SNR/10)),  c_n = sqrt(noise_pwr / 2)

(verified to rel err ~1.5e-7 against the fp32 jax reference).

Sharding: pure data-parallel over the leading batch dim N (1024) across
8 NeuronCores, 128 n-values per core.  All layouts keep (n,p) rows on
SBUF partitions and interleaved (freq, re/im) on the free axis, so every
HBM transfer is >= 512B-contiguous per row.

Per core (512 (n,p) rows = 4 partition-tiles of 128):
  - H, Hre-dup, HimS-dup via one K=16 matmul each against constant
    interleaved-DFT matrices (taps transposed on TensorE).
  - noise FFT: TensorE transpose per symbol -> PSUM -> SBUF -> K=128
    matmul against the constant noise-DFT matrix (c_n folded in).
  - H*X via two VectorE tensor_tensor passes with stride-0 broadcast
    APs (H broadcast over the 13 symbols), a lane-swap add, and a final
    add of the PSUM noise-FFT results.
"""

import numpy as np
from contextlib import ExitStack

import concourse.bass as bass
import concourse.bacc as bacc
import concourse.tile as tile
import concourse.mybir as mybir
from concourse.bass_utils import run_bass_kernel_spmd

# problem constants (hardcoded per contract)
N, P, S, M, K, L = 1024, 4, 12, 64, 16, 8
SNR, PWR = 20.0, 1.0
NOISE_PWR = PWR / (M * 10.0 ** (0.1 * SNR))  # 1/6400
C_N = float(np.sqrt(NOISE_PWR / 2.0))

NCORES = 8
NSH = N // NCORES            # 128 n per core
R = NSH * P                  # 512 (n,p) rows per core
RT = R // 128                # 4 row-tiles of 128 rows
NSYM = S + 1                 # 13 symbols (pilot + S payload)
W = 2 * M                    # 128 floats per symbol (interleaved re/im)
XW = NSYM * W                # 1664 floats per row across all symbols
MK2 = 2 * (M + K)            # 160 floats per noise symbol row

F32 = mybir.dt.float32
AF = mybir.ActivationFunctionType
ALU = mybir.AluOpType


def _constants():
    """Interleaved-complex DFT matrices.

    Layout convention: a complex vector z is stored as interleaved floats
    [re0, im0, re1, im1, ...].  A matmul  out = zT @ W2  with
    W2[(2m, 2m+1), (2f, 2f+1)] blocks implementing complex multiply by
    e^{-i phi} computes the DFT in interleaved form.
    """
    f = np.arange(M)
    # H = FFT64 of sqrt(prof/2)-scaled taps (L=8 taps, zero padded)
    phT = 2.0 * np.pi * np.outer(np.arange(L), f) / M     # (L, M)
    prof = np.exp(-np.arange(L, dtype=np.float64) / (L // 2))
    prof = prof / prof.sum()
    s = np.sqrt(prof / 2.0)[:, None]
    cT, sT = np.cos(phT), np.sin(phT)

    w2h = np.zeros((2 * L, W), np.float64)
    w2h[0::2, 0::2] = s * cT      # re <- u
    w2h[1::2, 0::2] = s * sT      # re <- v
    w2h[0::2, 1::2] = -s * sT     # im <- u
    w2h[1::2, 1::2] = s * cT      # im <- v

    # Hre duplicated into both lanes: [Hre, Hre] per f
    w2h_re = np.zeros((2 * L, W), np.float64)
    w2h_re[0::2, 0::2] = s * cT
    w2h_re[1::2, 0::2] = s * sT
    w2h_re[0::2, 1::2] = s * cT
    w2h_re[1::2, 1::2] = s * sT

    # HimS: even lane = +Him, odd lane = -Him  (Him = -u*sin + v*cos)
    w2h_ims = np.zeros((2 * L, W), np.float64)
    w2h_ims[0::2, 0::2] = -s * sT
    w2h_ims[1::2, 0::2] = s * cT
    w2h_ims[0::2, 1::2] = s * sT
    w2h_ims[1::2, 1::2] = -s * cT

    # noise FFT (64-point) with c_n folded in
    phM = 2.0 * np.pi * np.outer(np.arange(M), f) / M     # (M, M)
    cM, sM = np.cos(phM), np.sin(phM)
    w2n = np.zeros((W, W), np.float64)
    w2n[0::2, 0::2] = C_N * cM
    w2n[1::2, 0::2] = C_N * sM
    w2n[0::2, 1::2] = -C_N * sM
    w2n[1::2, 1::2] = C_N * cM

    ident = np.eye(128, dtype=np.float32)
    w2h3 = np.hstack([w2h, w2h_re, w2h_ims])   # (16, 384)
    return {
        "w2h3_d": w2h3.astype(np.float32),
        "w2n_d": w2n.astype(np.float32),
        "ident_d": ident,
    }


def _bcast13(ap):
    """[128, W] AP -> broadcast view [128, 13, W] (stride-0 symbol axis)."""
    return bass.AP(ap.tensor, ap.offset, [ap.ap[0], [0, NSYM], ap.ap[1]])


def build_nc():
    """Trace + compile the per-core kernel. Same program for all 8 cores."""
    nc = bacc.Bacc("TRN2", target_bir_lowering=False, debug=False)

    x_d = nc.dram_tensor("x_d", [R, S * W], F32, kind="ExternalInput").ap()
    pilot_d = nc.dram_tensor("pilot_d", [R, W], F32, kind="ExternalInput").ap()
    cofT_d = nc.dram_tensor("cofT_d", [2 * L, R], F32, kind="ExternalInput").ap()
    noise_d = nc.dram_tensor("noise_d", [R, NSYM, MK2], F32, kind="ExternalInput").ap()
    w2h3_d = nc.dram_tensor("w2h3_d", [2 * L, 3 * W], F32, kind="ExternalInput").ap()
    w2n_d = nc.dram_tensor("w2n_d", [W, W], F32, kind="ExternalInput").ap()
    ident_d = nc.dram_tensor("ident_d", [128, 128], F32, kind="ExternalInput").ap()

    sig_o = nc.dram_tensor("sig_o", [R, S * W], F32, kind="ExternalOutput").ap()
    pil_o = nc.dram_tensor("pil_o", [R, W], F32, kind="ExternalOutput").ap()
    h_o = nc.dram_tensor("h_o", [R, W], F32, kind="ExternalOutput").ap()

    with tile.TileContext(nc) as tc, ExitStack() as ctx:
        const = ctx.enter_context(tc.tile_pool(name="const", bufs=1))
        xpool = ctx.enter_context(tc.tile_pool(name="xpool", bufs=3))
        npool = ctx.enter_context(tc.tile_pool(name="npool", bufs=3))
        mpool = ctx.enter_context(tc.tile_pool(name="mpool", bufs=3))
        qpool = ctx.enter_context(tc.tile_pool(name="qpool", bufs=3))
        spool = ctx.enter_context(tc.tile_pool(name="spool", bufs=3))
        ps_big = ctx.enter_context(tc.tile_pool(name="ps_big", bufs=2, space="PSUM"))
        ps_xt = ctx.enter_context(tc.tile_pool(name="ps_xt", bufs=2, space="PSUM"))

        w2h3_t = const.tile([2 * L, 3 * W], F32)
        nc.sync.dma_start(w2h3_t[:], w2h3_d)
        w2n_t = const.tile([W, W], F32)
        nc.sync.dma_start(w2n_t[:], w2n_d)
        ident_t = const.tile([128, 128], F32)
        nc.sync.dma_start(ident_t[:], ident_d)

        # ---- channel taps: host-transposed [16, 512], one small DMA ---
        cofT_all = const.tile([2 * L, R], F32)
        nc.sync.dma_start(cofT_all[:], cofT_d)

        for rt in range(RT):
            r0 = rt * 128
            rsl = slice(r0, r0 + 128)

            # ---- loads -------------------------------------------------
            xfull = xpool.tile([128, XW], F32, tag="xfull")
            nc.sync.dma_start(xfull[:, 0:W], pilot_d[rsl, :])
            nc.sync.dma_start(xfull[:, W:XW], x_d[rsl, :])
            noise_t = npool.tile([128, XW], F32, tag="noise")
            nc.sync.dma_start(noise_t[:, 0:896], noise_d[rsl, 0:7, 2 * K:MK2])
            nc.sync.dma_start(noise_t[:, 896:XW], noise_d[rsl, 7:NSYM, 2 * K:MK2])
            # ---- pilot normalization (alpha) ---------------------------
            sq = spool.tile([128, W], F32, tag="sq")
            ss = spool.tile([128, 1], F32, tag="ss")
            nc.scalar.activation(sq[:], xfull[:, 0:W], AF.Square, accum_out=ss[:])
            st = spool.tile([128, 1], F32, tag="st")
            # alpha = 1/sqrt(sumsq/64)  (= sqrt(PWR/2)/sqrt(mean over 128))
            nc.scalar.activation(st[:], ss[:], AF.Sqrt, scale=1.0 / 64.0)
            al = spool.tile([128, 1], F32, tag="al")
            nc.vector.reciprocal(al[:], st[:])
            nc.scalar.activation(xfull[:, 0:W], xfull[:, 0:W], AF.Copy, scale=al[:])

            # ---- H chain (8-tap FFT via K=16 matmul) -------------------
            h3_ps = ps_xt.tile([128, 3 * W], F32, tag="xt_ps")
            nc.tensor.matmul(h3_ps[:], cofT_all[:, rsl], w2h3_t[:],
                             start=True, stop=True)
            h3_sb = spool.tile([128, 3 * W], F32, tag="h3_sb")
            nc.scalar.copy(h3_sb[:], h3_ps[:])
            nc.sync.dma_start(h_o[rsl, :], h3_sb[:, 0:W])
            hre_sb = h3_sb[:, W:2 * W]
            hims_sb = h3_sb[:, 2 * W:3 * W]

            # ---- noise FFT: transposes (2 psum halves) + matmuls -------
            xt_sb = spool.tile([128, XW], F32, tag="xt_sb")
            for j0, j1 in ((0, 7), (7, NSYM)):
                xt_ps = ps_xt.tile([128, 896], F32, tag="xt_ps")
                for j in range(j0, j1):
                    nc.tensor.transpose(
                        xt_ps[:, (j - j0) * 128:(j - j0 + 1) * 128],
                        noise_t[:, j * W:(j + 1) * W],
                        ident_t[:],
                    )
                nc.scalar.copy(xt_sb[:, j0 * 128:j1 * 128],
                               xt_ps[:, :(j1 - j0) * 128])
            prod_a = ps_big.tile([128, 896], F32, tag="prod")
            prod_b = ps_big.tile([128, 768], F32, tag="prod")
            for j in range(NSYM):
                pp, c0 = (prod_a, 0) if j < 7 else (prod_b, 896)
                nc.tensor.matmul(
                    pp[:, j * W - c0:(j + 1) * W - c0],
                    xt_sb[:, j * 128:(j + 1) * 128],
                    w2n_t[:],
                    start=True, stop=True,
                )

            # ---- H * X (interleaved complex mult, broadcast H over j) --
            m1 = mpool.tile([128, XW], F32, tag="m1")
            q = qpool.tile([128, XW], F32, tag="q")
            xv = xfull[:].rearrange("p (j w) -> p j w", w=W)
            m1v3 = m1[:].rearrange("p (j w) -> p j w", w=W)
            qv3 = q[:].rearrange("p (j w) -> p j w", w=W)
            nc.vector.tensor_tensor(m1v3, xv, _bcast13(hre_sb), ALU.mult)
            nc.vector.tensor_tensor(qv3, xv, _bcast13(hims_sb), ALU.mult)
            # m1 += swap(q):  even lane += q_odd, odd lane += q_even
            m1p = m1[:].rearrange("p (a b) -> p a b", b=2)
            qp = q[:].rearrange("p (a b) -> p a b", b=2)
            nc.vector.tensor_tensor(m1p[:, :, 0], m1p[:, :, 0], qp[:, :, 1], ALU.add)
            nc.vector.tensor_tensor(m1p[:, :, 1], m1p[:, :, 1], qp[:, :, 0], ALU.add)
            # out = m1 + noise_fft, per psum half (reuse q as out buffer)
            nc.vector.tensor_tensor(q[:, 0:896], m1[:, 0:896], prod_a[:], ALU.add)
            nc.sync.dma_start(pil_o[rsl, :], q[:, 0:W])
            nc.sync.dma_start(sig_o[rsl, 0:768], q[:, W:896])
            nc.vector.tensor_tensor(q[:, 896:XW], m1[:, 896:XW], prod_b[:], ALU.add)
            nc.sync.dma_start(sig_o[rsl, 768:S * W], q[:, 896:XW])

    nc.compile()
    return nc


_CACHE = {}


def _get_nc():
    if "nc" not in _CACHE:
        _CACHE["nc"] = build_nc()
        _CACHE["consts"] = _constants()
    return _CACHE["nc"], _CACHE["consts"]


def make_in_maps(x, pilot_raw, cof_unit, noise_unit):
    consts = _get_nc()[1]
    in_maps = []
    for c in range(NCORES):
        sl = slice(c * NSH, (c + 1) * NSH)
        in_maps.append({
            "x_d": np.ascontiguousarray(x[sl]).reshape(R, S * W),
            "pilot_d": np.ascontiguousarray(pilot_raw[sl]).reshape(R, W),
            "cofT_d": np.ascontiguousarray(
                cof_unit[sl].reshape(R, 2 * L).T),
            "noise_d": np.ascontiguousarray(noise_unit[sl]).reshape(R, NSYM, MK2),
            **consts,
        })
    return in_maps


def assemble(results):
    pil = np.concatenate([r["pil_o"] for r in results], axis=0)
    sig = np.concatenate([r["sig_o"] for r in results], axis=0)
    h = np.concatenate([r["h_o"] for r in results], axis=0)
    info_pilot = pil.reshape(N, P, 1, M, 2)
    info_sig = sig.reshape(N, P, S, M, 2)
    h_true = h.reshape(N, P, M, 2)
    return info_pilot, info_sig, h_true, np.float32(NOISE_PWR)


def kernel(x, pilot_raw, cof_unit, noise_unit, _trace=False, _trace_kwargs=None):
    x = np.asarray(x, dtype=np.float32)
    pilot_raw = np.asarray(pilot_raw, dtype=np.float32)
    cof_unit = np.asarray(cof_unit, dtype=np.float32)
    noise_unit = np.asarray(noise_unit, dtype=np.float32)

    nc, _ = _get_nc()
    in_maps = make_in_maps(x, pilot_raw, cof_unit, noise_unit)
    res = run_bass_kernel_spmd(
        nc, in_maps, core_ids=list(range(NCORES)),
        trace=_trace, **(_trace_kwargs or {}),
    )
    out = assemble(res.results)
    if _trace:
        return out, res
    return out
